# revision 26
# baseline (speedup 1.0000x reference)
"""Trainium2 Bass kernel for CrossAttention (B=4, QL=KL=2048, D=1024, fp32).

reference:
    query = hidden_states @ Wq                      # [B, QL, D]
    kv    = decoder_hidden_states @ Wkv             # [B, KL, 2D]
    key, value = split(kv, 2, axis=-1)
    scores = einsum('bqd,bkd->bqk', query, key) / sqrt(D)
    w = softmax(scores, axis=-1)
    out = einsum('bqk,bkd->bqd', w, value)          # [B, QL, D]

Sharding: 8 cores = batch(4) x pair(2).  Core h of a pair owns query rows
[h*1024, (h+1)*1024) AND computes the K/V projection only for keys
[h*1024, (h+1)*1024) of its batch — no duplicated KV work.  The pair then
exchanges K/V halves with two HBM AllGathers (replica groups [0,1],[2,3],
[4,5],[6,7]) which overlap with the V/Q projections on the PE.

All matmul operands are bf16 (same 1 row/cycle PE rate as f32r, half the
DMA bytes and SBUF footprint) with f32 PSUM accumulation.  Scores are
computed TRANSPOSED (S^T[k,q] = K^T-chunk^T-stationary @ Q^T-moving) so
exp(S^T) = P^T feeds the AV matmul directly as the stationary operand —
no DVE transposes at all.  Softmax uses no max-subtraction (scores are
~N(0,1)); row sums are accumulated over k-tiles with DVE adds plus one
tiny ones-matmul per q-tile to reduce over partitions.

Consecutive matmuls that share a stationary operand are emitted with
ldweights=False on the second one (walrus then skips the redundant
LDWEIGHTS, which otherwise serializes ~59ns per matmul).

This walrus build allows only ONE embedded semaphore wait per hardware
instruction; legalize_waits() splits any extra waits onto injected
same-engine NOPs after Tile scheduling.
"""

import sys

if "/opt/trn_rl_repo" not in sys.path:
    sys.path.insert(0, "/opt/trn_rl_repo")

import numpy as np
import ml_dtypes

import bass_rust
import concourse.bass as bass
import concourse.mybir as mybir
import concourse.tile as tile
from concourse.bass_utils import run_bass_kernel_spmd

F32 = mybir.dt.float32
BF16 = mybir.dt.bfloat16
EXP = mybir.ActivationFunctionType.Exp
ACOPY = mybir.ActivationFunctionType.Copy

N_CORES = 8
B, QL, KL, D = 4, 2048, 2048, 1024
PAIRS = [[0, 1], [2, 3], [4, 5], [6, 7]]

ELIDE_LDW = True     # skip LDWEIGHTS when stationary repeats
MIXED_ADD = True     # DVE add with bf16 in1 / f32 in0+out for row sums
N_WARM = 14          # PE warmup matmuls during the initial DMA wave


def legalize_waits(nc, max_waits=1):
    """TRN2 instructions embed at most one semaphore wait.  Move excess waits
    emitted by Tile onto same-engine NOPs inserted just before the owning
    instruction (engine FIFO makes this semantically identical)."""
    cnt = 0
    for fn in nc.m.functions:
        for bb in fn.blocks:
            out = []
            changed = False
            for ins in bb.instructions:
                si = ins.sync_info
                if si is not None and si.on_wait and len(si.on_wait) > max_waits:
                    waits = list(si.on_wait)
                    for w in waits[:-max_waits]:
                        cnt += 1
                        nop = bass_rust.InstNoOp(name=f"I-wfix-{cnt}")
                        nop.engine = ins.engine
                        nop.sync_info = mybir.SyncInfo(on_wait=[w], on_update=[])
                        out.append(nop)
                    ins.sync_info = mybir.SyncInfo(
                        on_wait=waits[-max_waits:],
                        on_update=list(si.on_update or []),
                    )
                    changed = True
                out.append(ins)
            if changed:
                bb.instructions = out
    return cnt


def elide_redundant_ldweights(nc):
    """Tile legalization splits each matmul into InstLdweights + InstMatmult.
    When consecutive PE weight loads read the IDENTICAL physical AP (our loop
    orders pair matmuls sharing a stationary operand), the second load is
    redundant: the PE array still holds those weights, and matmuls do not
    disturb them.  Remove such loads (only when they carry no semaphore
    waits; any on_update is merged into the paired matmul).  Saves the
    ~55ns/matmul LDWEIGHTS serialization for half the matmuls."""
    removed = 0
    for fn in nc.m.functions:
        for bb in fn.blocks:
            out = []
            last_w = None
            for ins in bb.instructions:
                if getattr(ins, "engine", None) != mybir.EngineType.PE:
                    out.append(ins)
                    continue
                if isinstance(ins, mybir.InstLdweights):
                    key = (
                        str(ins.ins[0]),
                        ins.perf_mode,
                        ins.is_transpose,
                        ins.tile_position,
                        ins.tile_size,
                    )
                    si = ins.sync_info
                    no_sync = si is None or (not si.on_wait and not si.on_update)
                    if key == last_w and no_sync:
                        removed += 1
                        continue
                    last_w = key
                    out.append(ins)
                elif isinstance(ins, mybir.InstMatmult):
                    out.append(ins)
                else:
                    # any other PE instruction invalidates the cached weights
                    last_w = None
                    out.append(ins)
            bb.instructions = out
    return removed


def build_attention(nc, QS, KLp, Dp, scale):
    DS = Dp // 128       # contraction 128-chunks of d
    NDO = Dp // 128      # output-d 128-chunks
    NQC = QS // 512      # q 512-chunks
    NQT = QS // 128      # q 128-tiles
    KH = KLp // 2        # keys owned by this core
    NKC = KH // 512      # own-k 512-chunks
    NKTH = KH // 128     # own-k 128-tiles
    NKT = KLp // 128     # global k 128-tiles
    NDC = Dp // 512      # d 512-chunks

    # inputs, laid out exactly as their SBUF destinations (bf16)
    hsT = nc.declare_dram_parameter("hsT", [128, DS, QS], BF16, isOutput=False)
    decT = nc.declare_dram_parameter("decT", [128, DS, KH], BF16, isOutput=False)
    wq = nc.declare_dram_parameter("wq", [128, NDO, DS, 128], BF16, isOutput=False)
    wkvlo = nc.declare_dram_parameter("wkvlo", [128, NDO, DS, 128], BF16, isOutput=False)
    wkvhi = nc.declare_dram_parameter("wkvhi", [128, DS, Dp], BF16, isOutput=False)
    out = nc.declare_dram_parameter("out", [QS, Dp], F32, isOutput=True)

    with tile.TileContext(nc) as tc:
        pools = []

        def enter(cm):
            pools.append(cm)
            return cm.__enter__()

        def close(cm):
            pools.remove(cm)
            cm.__exit__(None, None, None)

        # right stack: long-lived
        constp = enter(tc.tile_pool(name="const", bufs=1, side="right"))
        ktp = enter(tc.tile_pool(name="ktp", bufs=1, side="right"))
        vp = enter(tc.tile_pool(name="vp", bufs=1, side="right"))
        qtp = enter(tc.tile_pool(name="qtp", bufs=1, side="right"))
        rcp = enter(tc.tile_pool(name="rcp", bufs=2, side="right"))
        ostp = enter(tc.tile_pool(name="ost", bufs=2, side="right"))
        dramp = enter(tc.tile_pool(name="dram", bufs=1, space="DRAM"))

        # left stack: transient, opened in reverse order of closing
        hstp_cm = tc.tile_pool(name="hstp", bufs=1)
        wqp_cm = tc.tile_pool(name="wqp", bufs=1)
        whip_cm = tc.tile_pool(name="whip", bufs=1)
        decp_cm = tc.tile_pool(name="decp", bufs=1)
        stgp_cm = tc.tile_pool(name="stgp", bufs=6)
        wlop_cm = tc.tile_pool(name="wlop", bufs=1)
        warmp_cm = tc.tile_pool(name="warmp", bufs=1)
        hstp = enter(hstp_cm)
        wqp = enter(wqp_cm)
        whip = enter(whip_cm)
        decp = enter(decp_cm)
        stgp = enter(stgp_cm)
        wlop = enter(wlop_cm)
        warmp = enter(warmp_cm)

        psproj_cm = tc.tile_pool(name="psproj", bufs=6, space="PSUM")
        warmps_cm = tc.tile_pool(name="warmps", bufs=1, space="PSUM")
        psproj = enter(psproj_cm)
        warmps = enter(warmps_cm)

        # collective bounce buffers (DRAM); KT exchange split by own-k chunk
        # so the first half kicks off at the midpoint of the KT projection
        cc_in_kt = [
            dramp.tile([NDO, 128, 512], BF16, name=f"cc_in_kt{i}")
            for i in range(NKC)
        ]
        cc_out_kt = [
            dramp.tile([2, NDO, 128, 512], BF16, name=f"cc_out_kt{i}")
            for i in range(NKC)
        ]
        cc_in_v = dramp.tile([NKTH, NDC, 128, 512], BF16, name="cc_in_v")
        cc_out_v = dramp.tile([2, NKTH, NDC, 128, 512], BF16, name="cc_out_v")

        # constants
        ones = constp.tile([128, 1], F32)
        nc.gpsimd.memset(ones[:], 1.0)

        # long-lived SBUF tensors
        KT = ktp.tile([128, DS, KLp], BF16, tag="KT")     # [d-in-chunk, di, k]
        V = vp.tile([128, NKT, Dp], BF16, tag="V")        # [k-in-tile, kt, d]
        QT = qtp.tile([128, DS, QS], BF16, tag="QT")      # [d-in-chunk, di, q]

        # ---- transient input tiles + critical-first DMA issue order --------
        wlo_s = wlop.tile([128, NDO, DS, 128], BF16, tag="wlo")
        dec_s = decp.tile([128, DS, KH], BF16, tag="dec")
        whi_s = whip.tile([128, DS, Dp], BF16, tag="whi")
        wq_s = wqp.tile([128, NDO, DS, 128], BF16, tag="wq")
        hs_s = hstp.tile([128, DS, QS], BF16, tag="hs")
        # critical-first: the kc=0 pass of Phase 1 needs dec[:, :, 0:512]
        # plus wkvlo only; stream the rest behind it
        nc.sync.dma_start(dec_s[:, :, 0:KH // 2], decT[:, :, 0:KH // 2])
        nc.sync.dma_start(wlo_s[:, 0:NDO // 2], wkvlo[:, 0:NDO // 2])
        nc.sync.dma_start(wlo_s[:, NDO // 2:], wkvlo[:, NDO // 2:])
        nc.sync.dma_start(dec_s[:, :, KH // 2:], decT[:, :, KH // 2:])
        nc.sync.dma_start(whi_s[:], wkvhi[:])
        nc.sync.dma_start(wq_s[:], wq[:])
        nc.sync.dma_start(hs_s[:], hsT[:])

        # HAM warmup: keep the PE busy during the initial DMA wave so the
        # clock gate and p-state are fully up when the first real matmul
        # issues.  The warm tile is read uninitialized (results are never
        # consumed); no memset so the PE can start the instant its queue is
        # live.  Also pre-warm the ACT engine (exp table load) here.
        warm = warmp.tile([128, 640], BF16)
        nc.gpsimd.memset(warm[:], 0.5)
        warm_ps = warmps.tile([128, 512], F32)
        for _ in range(N_WARM):
            nc.tensor.matmul(
                warm_ps[:], warm[:, 0:128], warm[:, 128:640],
                start=True, stop=True, skip_group_check=True,
            )
        act_warm = constp.tile([128, 1], F32, name="act_warm")
        nc.scalar.activation(act_warm[:], ones[:], EXP, bias=0.0, scale=1.0)
        close(warmps_cm)
        close(warmp_cm)

        # ---- Phase 1: KT-own = Wkv_lo^T @ decT_own -> stage -> cc_in ------
        # kc-outer: the kc=0 pass depends only on the first half of dec, so
        # compute starts as soon as ~1.25MB of input has landed, and the
        # first KT exchange kicks off at the phase midpoint.
        for kc in range(NKC):
            for do in range(NDO):
                ps = psproj.tile([128, 512], F32, tag="psp", name=f"psk{kc}_{do}")
                for di in range(DS):
                    nc.tensor.matmul(
                        ps[:], wlo_s[:, do, di, :],
                        dec_s[:, di, kc * 512 : (kc + 1) * 512],
                        start=(di == 0), stop=(di == DS - 1),
                    )
                st = stgp.tile([128, 512], BF16, tag="stg", name=f"stk{kc}_{do}")
                nc.vector.tensor_copy(st[:], ps[:])
                nc.sync.dma_start(cc_in_kt[kc][do], st[:])
            nc.gpsimd.collective_compute(
                "AllGather", mybir.AluOpType.bypass, replica_groups=PAIRS,
                ins=[cc_in_kt[kc].opt()], outs=[cc_out_kt[kc].opt()],
            )
            # readback both ranks (own half re-read too: keeps the program
            # rank-symmetric and the key order [rank0|rank1])
            for r in range(2):
                k0 = r * KH + kc * 512
                nc.sync.dma_start(
                    KT[:, :, k0 : k0 + 512],
                    cc_out_kt[kc][r].rearrange("d p x -> p d x"),
                )
        close(wlop_cm)

        # ---- Phase 2: V-own = decT_own^T @ Wkv_hi -> stage -> cc_in -------
        for kt in range(NKTH):
            pss = [psproj.tile([128, 512], F32, tag="psp", name=f"psv{kt}_{i}") for i in range(NDC)]
            for di in range(DS):
                for dc in range(NDC):
                    mm = nc.tensor.matmul(
                        pss[dc][:], dec_s[:, di, kt * 128 : (kt + 1) * 128],
                        whi_s[:, di, dc * 512 : (dc + 1) * 512],
                        start=(di == 0), stop=(di == DS - 1),
                    )
                    if ELIDE_LDW and dc > 0:
                        mm.ins.ldweights = False
            for dc in range(NDC):
                st = stgp.tile([128, 512], BF16, tag="stg", name=f"stv{kt}_{dc}")
                nc.vector.tensor_copy(st[:], pss[dc][:])
                nc.sync.dma_start(cc_in_v[kt, dc], st[:])
        nc.gpsimd.collective_compute(
            "AllGather", mybir.AluOpType.bypass, replica_groups=PAIRS,
            ins=[cc_in_v.opt()], outs=[cc_out_v.opt()],
        )
        for r in range(2):
            for dc in range(NDC):
                nc.sync.dma_start(
                    V[:, r * NKTH : (r + 1) * NKTH, dc * 512 : (dc + 1) * 512],
                    cc_out_v[r, :, dc].rearrange("t p x -> p t x"),
                )
        close(stgp_cm)
        close(decp_cm)
        close(whip_cm)

        # ---- Phase 3: QT = Wq^T @ hsT (stays in SBUF) ---------------------
        for do in range(NDO):
            pss = [psproj.tile([128, 512], F32, tag="psp", name=f"psq{do}_{i}") for i in range(NQC)]
            for di in range(DS):
                for qc in range(NQC):
                    mm = nc.tensor.matmul(
                        pss[qc][:], wq_s[:, do, di, :],
                        hs_s[:, di, qc * 512 : (qc + 1) * 512],
                        start=(di == 0), stop=(di == DS - 1),
                    )
                    if ELIDE_LDW and qc > 0:
                        mm.ins.ldweights = False
            for qc in range(NQC):
                nc.vector.tensor_copy(
                    QT[:, do, qc * 512 : (qc + 1) * 512], pss[qc][:]
                )
        close(wqp_cm)
        close(hstp_cm)
        close(psproj_cm)

        # ---- Phase 4: scores^T + exp + row-sum partials -------------------
        ptp = enter(tc.tile_pool(name="ptp", bufs=2, side="right"))
        lap = enter(tc.tile_pool(name="lap", bufs=2, side="right"))
        ps_sc_cm = tc.tile_pool(name="ps_sc", bufs=4, space="PSUM")
        ps_sc = enter(ps_sc_cm)
        PTs = [ptp.tile([128, NKT, 512], BF16, tag="pt", name=f"PT{qc}")
               for qc in range(NQC)]
        Las = [lap.tile([128, 512], F32, tag="la", name=f"La{qc}")
               for qc in range(NQC)]
        for kt in range(NKT):
            pss = [ps_sc.tile([128, 512], F32, tag="ps_sc", name=f"pss{kt}_{i}") for i in range(NQC)]
            for di in range(DS):
                for qc in range(NQC):
                    mm = nc.tensor.matmul(
                        pss[qc][:], KT[:, di, kt * 128 : (kt + 1) * 128],
                        QT[:, di, qc * 512 : (qc + 1) * 512],
                        start=(di == 0), stop=(di == DS - 1),
                    )
                    if ELIDE_LDW and qc > 0:
                        mm.ins.ldweights = False
            for qc in range(NQC):
                nc.scalar.activation(
                    PTs[qc][:, kt, :], pss[qc][:], EXP,
                    bias=0.0, scale=float(scale),
                )
                if kt == 0:
                    nc.vector.tensor_copy(Las[qc][:], PTs[qc][:, kt, :])
                else:
                    nc.vector.tensor_tensor(
                        Las[qc][:], Las[qc][:], PTs[qc][:, kt, :],
                        mybir.AluOpType.add,
                    )

        # ---- Phase 5: AV + normalize per q-tile ---------------------------
        ps_av_cm = tc.tile_pool(name="ps_av", bufs=3, space="PSUM")
        psl_cm = tc.tile_pool(name="psl", bufs=1, space="PSUM")
        ps_av = enter(ps_av_cm)
        psl = enter(psl_cm)
        for qg in range(NQT):
            qc, qt = qg // (NQT // NQC), qg % (NQT // NQC)
            lt = psl.tile([128, 1], F32, tag="psl", name=f"lt{qg}")
            nc.tensor.matmul(
                lt[:], Las[qc][:, qt * 128 : (qt + 1) * 128], ones[:],
                start=True, stop=True,
            )
            rc = rcp.tile([128, 1], F32, tag="rc", name=f"rc{qg}")
            nc.vector.reciprocal(rc[:], lt[:])
            avs = [ps_av.tile([128, 512], F32, tag="ps_av", name=f"av{qg}_{i}") for i in range(NDC)]
            for kt in range(NKT):
                for dc in range(NDC):
                    mm = nc.tensor.matmul(
                        avs[dc][:], PTs[qc][:, kt, qt * 128 : (qt + 1) * 128],
                        V[:, kt, dc * 512 : (dc + 1) * 512],
                        start=(kt == 0), stop=(kt == NKT - 1),
                    )
                    if ELIDE_LDW and dc > 0:
                        mm.ins.ldweights = False
            ot = ostp.tile([128, Dp], F32, tag="ost", name=f"ot{qg}")
            for dc in range(NDC):
                nc.scalar.activation(
                    ot[:, dc * 512 : (dc + 1) * 512], avs[dc][:],
                    ACOPY, bias=0.0, scale=rc[:],
                )
            nc.sync.dma_start(out[qg * 128 : (qg + 1) * 128, :], ot[:])

        for cm in list(reversed(pools)):
            close(cm)

    if ELIDE_LDW:
        elide_redundant_ldweights(nc)
    legalize_waits(nc)
    return nc


def _bf16(x):
    return np.asarray(x, dtype=ml_dtypes.bfloat16)


def prepare_in_maps(hidden_states, decoder_hidden_states, Wq, Wkv):
    hs = np.asarray(hidden_states, dtype=np.float32)
    dec = np.asarray(decoder_hidden_states, dtype=np.float32)
    Wq = np.asarray(Wq, dtype=np.float32)
    Wkv = np.asarray(Wkv, dtype=np.float32)
    QS = QL // 2
    KH = KL // 2
    DS = D // 128
    NDO = D // 128

    # wq/wkvlo: [p, do, di, o] = W[di*128+p, do*128+o]
    def pack_st(W):
        r = W.reshape(DS, 128, NDO, 128).transpose(1, 2, 0, 3)
        return _bf16(np.ascontiguousarray(r))

    wq_p = pack_st(Wq)
    wkvlo_p = pack_st(Wkv[:, :D])
    # wkvhi: [p, di, j] = Wkv[di*128+p, D+j]
    wkvhi_p = _bf16(np.ascontiguousarray(
        Wkv[:, D:].reshape(DS, 128, D).transpose(1, 0, 2)))

    def pack_xT(x):
        # [N, D] -> [p, di, n] = x[n, di*128+p]
        n = x.shape[0]
        r = x.reshape(n, DS, 128).transpose(2, 1, 0)
        return _bf16(np.ascontiguousarray(r))

    in_maps = []
    for c in range(N_CORES):
        b, h = c // 2, c % 2
        in_maps.append({
            "hsT": pack_xT(hs[b, h * QS : (h + 1) * QS]),
            "decT": pack_xT(dec[b, h * KH : (h + 1) * KH]),
            "wq": wq_p,
            "wkvlo": wkvlo_p,
            "wkvhi": wkvhi_p,
        })
    return in_maps


def kernel(hidden_states, decoder_hidden_states, Wq, Wkv):
    QS = QL // 2
    scale = 1.0 / float(np.sqrt(D))

    nc = bass.Bass()
    build_attention(nc, QS, KL, D, scale)
    in_maps = prepare_in_maps(hidden_states, decoder_hidden_states, Wq, Wkv)

    res = run_bass_kernel_spmd(nc, in_maps, list(range(N_CORES)))

    out = np.empty((B, QL, D), dtype=np.float32)
    for c in range(N_CORES):
        b, h = c // 2, c % 2
        out[b, h * QS : (h + 1) * QS] = res.results[c]["out"]
    return out


# revision 32
# speedup vs baseline: 1.0480x; 1.0480x over previous
"""Trainium2 Bass kernel for CrossAttention (B=4, QL=KL=2048, D=1024, fp32).

reference:
    query = hidden_states @ Wq                      # [B, QL, D]
    kv    = decoder_hidden_states @ Wkv             # [B, KL, 2D]
    key, value = split(kv, 2, axis=-1)
    scores = einsum('bqd,bkd->bqk', query, key) / sqrt(D)
    w = softmax(scores, axis=-1)
    out = einsum('bqk,bkd->bqd', w, value)          # [B, QL, D]

Sharding: 8 cores = batch(4) x pair(2).  Core h of a pair owns query rows
[h*1024, (h+1)*1024) AND computes the K/V projection only for keys
[h*1024, (h+1)*1024) of its batch — no duplicated KV work.  The pair then
exchanges K/V halves with two HBM AllGathers (replica groups [0,1],[2,3],
[4,5],[6,7]) which overlap with the V/Q projections on the PE.

All matmul operands are bf16 (same 1 row/cycle PE rate as f32r, half the
DMA bytes and SBUF footprint) with f32 PSUM accumulation.  Scores are
computed TRANSPOSED (S^T[k,q] = K^T-chunk^T-stationary @ Q^T-moving) so
exp(S^T) = P^T feeds the AV matmul directly as the stationary operand —
no DVE transposes at all.  Softmax uses no max-subtraction (scores are
~N(0,1)); row sums are accumulated over k-tiles with DVE adds plus one
tiny ones-matmul per q-tile to reduce over partitions.

Consecutive matmuls that share a stationary operand are emitted with
ldweights=False on the second one (walrus then skips the redundant
LDWEIGHTS, which otherwise serializes ~59ns per matmul).

This walrus build allows only ONE embedded semaphore wait per hardware
instruction; legalize_waits() splits any extra waits onto injected
same-engine NOPs after Tile scheduling.
"""

import sys

if "/opt/trn_rl_repo" not in sys.path:
    sys.path.insert(0, "/opt/trn_rl_repo")

import numpy as np
import ml_dtypes

import bass_rust
import concourse.bass as bass
import concourse.mybir as mybir
import concourse.tile as tile
from concourse.bass_utils import run_bass_kernel_spmd

F32 = mybir.dt.float32
BF16 = mybir.dt.bfloat16
EXP = mybir.ActivationFunctionType.Exp
ACOPY = mybir.ActivationFunctionType.Copy

N_CORES = 8
B, QL, KL, D = 4, 2048, 2048, 1024
PAIRS = [[0, 1], [2, 3], [4, 5], [6, 7]]

ELIDE_LDW = True     # skip LDWEIGHTS when stationary repeats
MIXED_ADD = True     # DVE add with bf16 in1 / f32 in0+out for row sums
N_WARM = 8           # PE warmup matmuls during the initial DMA wave


def legalize_waits(nc, max_waits=1):
    """TRN2 instructions embed at most one semaphore wait.  Move excess waits
    emitted by Tile onto same-engine NOPs inserted just before the owning
    instruction (engine FIFO makes this semantically identical)."""
    cnt = 0
    for fn in nc.m.functions:
        for bb in fn.blocks:
            out = []
            changed = False
            for ins in bb.instructions:
                si = ins.sync_info
                if si is not None and si.on_wait and len(si.on_wait) > max_waits:
                    waits = list(si.on_wait)
                    for w in waits[:-max_waits]:
                        cnt += 1
                        nop = bass_rust.InstNoOp(name=f"I-wfix-{cnt}")
                        nop.engine = ins.engine
                        nop.sync_info = mybir.SyncInfo(on_wait=[w], on_update=[])
                        out.append(nop)
                    ins.sync_info = mybir.SyncInfo(
                        on_wait=waits[-max_waits:],
                        on_update=list(si.on_update or []),
                    )
                    changed = True
                out.append(ins)
            if changed:
                bb.instructions = out
    return cnt


def elide_redundant_ldweights(nc):
    """Tile legalization splits each matmul into InstLdweights + InstMatmult.
    When consecutive PE weight loads read the IDENTICAL physical AP (our loop
    orders pair matmuls sharing a stationary operand), the second load is
    redundant: the PE array still holds those weights, and matmuls do not
    disturb them.  Remove such loads (only when they carry no semaphore
    waits; any on_update is merged into the paired matmul).  Saves the
    ~55ns/matmul LDWEIGHTS serialization for half the matmuls."""
    removed = 0
    for fn in nc.m.functions:
        for bb in fn.blocks:
            out = []
            last_w = None
            for ins in bb.instructions:
                if getattr(ins, "engine", None) != mybir.EngineType.PE:
                    out.append(ins)
                    continue
                if isinstance(ins, mybir.InstLdweights):
                    key = (
                        str(ins.ins[0]),
                        ins.perf_mode,
                        ins.is_transpose,
                        ins.tile_position,
                        ins.tile_size,
                    )
                    si = ins.sync_info
                    no_sync = si is None or (not si.on_wait and not si.on_update)
                    if key == last_w and no_sync:
                        removed += 1
                        continue
                    last_w = key
                    out.append(ins)
                elif isinstance(ins, mybir.InstMatmult):
                    out.append(ins)
                else:
                    # any other PE instruction invalidates the cached weights
                    last_w = None
                    out.append(ins)
            bb.instructions = out
    return removed


def build_attention(nc, QS, KLp, Dp, scale):
    DS = Dp // 128       # contraction 128-chunks of d
    NDO = Dp // 128      # output-d 128-chunks
    NQC = QS // 512      # q 512-chunks
    NQT = QS // 128      # q 128-tiles
    KH = KLp // 2        # keys owned by this core
    NKC = KH // 512      # own-k 512-chunks
    NKTH = KH // 128     # own-k 128-tiles
    NKT = KLp // 128     # global k 128-tiles
    NDC = Dp // 512      # d 512-chunks

    # inputs, laid out exactly as their SBUF destinations (bf16)
    hsT = nc.declare_dram_parameter("hsT", [128, DS, QS], BF16, isOutput=False)
    # dec is kc-major so each half is one fat-descriptor DMA (128 x 8KB)
    decT = nc.declare_dram_parameter("decT", [NKC, 128, DS, 512], BF16, isOutput=False)
    wq = nc.declare_dram_parameter("wq", [128, NDO, DS, 128], BF16, isOutput=False)
    wkvlo = nc.declare_dram_parameter("wkvlo", [128, NDO, DS, 128], BF16, isOutput=False)
    wkvhi = nc.declare_dram_parameter("wkvhi", [128, DS, Dp], BF16, isOutput=False)
    out = nc.declare_dram_parameter("out", [QS, Dp], F32, isOutput=True)

    with tile.TileContext(nc) as tc:
        pools = []

        def enter(cm):
            pools.append(cm)
            return cm.__enter__()

        def close(cm):
            pools.remove(cm)
            cm.__exit__(None, None, None)

        # right stack: long-lived
        constp = enter(tc.tile_pool(name="const", bufs=1, side="right"))
        ktp = enter(tc.tile_pool(name="ktp", bufs=1, side="right"))
        vp = enter(tc.tile_pool(name="vp", bufs=1, side="right"))
        qtp = enter(tc.tile_pool(name="qtp", bufs=1, side="right"))
        rcp = enter(tc.tile_pool(name="rcp", bufs=2, side="right"))
        ostp = enter(tc.tile_pool(name="ost", bufs=2, side="right"))
        dramp = enter(tc.tile_pool(name="dram", bufs=1, space="DRAM"))

        # left stack: transient, opened in reverse order of closing
        hstp_cm = tc.tile_pool(name="hstp", bufs=1)
        wqp_cm = tc.tile_pool(name="wqp", bufs=1)
        whip_cm = tc.tile_pool(name="whip", bufs=1)
        decp_cm = tc.tile_pool(name="decp", bufs=1)
        stgp_cm = tc.tile_pool(name="stgp", bufs=6)
        wlop_cm = tc.tile_pool(name="wlop", bufs=1)
        warmp_cm = tc.tile_pool(name="warmp", bufs=1)
        hstp = enter(hstp_cm)
        wqp = enter(wqp_cm)
        whip = enter(whip_cm)
        decp = enter(decp_cm)
        stgp = enter(stgp_cm)
        wlop = enter(wlop_cm)
        warmp = enter(warmp_cm)

        psproj_cm = tc.tile_pool(name="psproj", bufs=6, space="PSUM")
        warmps_cm = tc.tile_pool(name="warmps", bufs=1, space="PSUM")
        psproj = enter(psproj_cm)
        warmps = enter(warmps_cm)

        # collective bounce buffers (DRAM); KT exchange split by own-k chunk
        # so the first half kicks off at the midpoint of the KT projection
        cc_in_kt = [
            dramp.tile([NDO, 128, 512], BF16, name=f"cc_in_kt{i}")
            for i in range(NKC)
        ]
        cc_out_kt = [
            dramp.tile([2, NDO, 128, 512], BF16, name=f"cc_out_kt{i}")
            for i in range(NKC)
        ]
        cc_in_v = dramp.tile([NKTH, NDC, 128, 512], BF16, name="cc_in_v")
        cc_out_v = dramp.tile([2, NKTH, NDC, 128, 512], BF16, name="cc_out_v")

        # constants
        ones = constp.tile([128, 1], F32)
        nc.gpsimd.memset(ones[:], 1.0)

        # long-lived SBUF tensors
        KT = ktp.tile([128, DS, KLp], BF16, tag="KT")     # [d-in-chunk, di, k]
        V = vp.tile([128, NKT, Dp], BF16, tag="V")        # [k-in-tile, kt, d]
        QT = qtp.tile([128, DS, QS], BF16, tag="QT")      # [d-in-chunk, di, q]

        # ---- transient input tiles + critical-first DMA issue order --------
        wlo_s = wlop.tile([128, NDO, DS, 128], BF16, tag="wlo")
        dec_s = decp.tile([128, NKC, DS, 512], BF16, tag="dec")
        whi_s = whip.tile([128, DS, Dp], BF16, tag="whi")
        wq_s = wqp.tile([128, NDO, DS, 128], BF16, tag="wq")
        hs_s = hstp.tile([128, DS, QS], BF16, tag="hs")
        # critical-first: the kc=0 pass of Phase 1 needs dec half 0 plus
        # wkvlo only; stream the rest behind it.  Input loads ride the
        # SCALAR engine's DMA queue so they never block the stage/readback
        # DMAs on the sync queue.
        nc.scalar.dma_start(dec_s[:, 0], decT[0])
        nc.scalar.dma_start(wlo_s[:, 0:NDO // 2], wkvlo[:, 0:NDO // 2])
        nc.scalar.dma_start(wlo_s[:, NDO // 2:], wkvlo[:, NDO // 2:])
        nc.scalar.dma_start(dec_s[:, 1], decT[1])
        nc.scalar.dma_start(whi_s[:], wkvhi[:])
        nc.scalar.dma_start(wq_s[:], wq[:])
        nc.scalar.dma_start(hs_s[:], hsT[:])

        # HAM warmup: keep the PE busy during the initial DMA wave so the
        # clock gate and p-state are fully up when the first real matmul
        # issues.  The warm tile is read uninitialized (results are never
        # consumed); no memset so the PE can start the instant its queue is
        # live.  Also pre-warm the ACT engine (exp table load) here.
        warm = warmp.tile([128, 640], BF16)
        nc.gpsimd.memset(warm[:], 0.5)
        warm_ps = warmps.tile([128, 512], F32)
        for _ in range(N_WARM):
            nc.tensor.matmul(
                warm_ps[:], warm[:, 0:128], warm[:, 128:640],
                start=True, stop=True, skip_group_check=True,
            )
        act_warm = constp.tile([128, 1], F32, name="act_warm")
        nc.scalar.activation(act_warm[:], ones[:], EXP, bias=0.0, scale=1.0)
        close(warmps_cm)
        close(warmp_cm)

        # ---- Phase 1: KT-own = Wkv_lo^T @ decT_own -> stage -> cc_in ------
        # kc-outer: the kc=0 pass depends only on the first half of dec, so
        # compute starts as soon as ~1.25MB of input has landed, and the
        # first KT exchange kicks off at the phase midpoint.
        for kc in range(NKC):
            for do in range(NDO):
                ps = psproj.tile([128, 512], F32, tag="psp", name=f"psk{kc}_{do}")
                for di in range(DS):
                    nc.tensor.matmul(
                        ps[:], wlo_s[:, do, di, :],
                        dec_s[:, kc, di, :],
                        start=(di == 0), stop=(di == DS - 1),
                    )
                st = stgp.tile([128, 512], BF16, tag="stg", name=f"stk{kc}_{do}")
                nc.vector.tensor_copy(st[:], ps[:])
                nc.sync.dma_start(cc_in_kt[kc][do], st[:])
            nc.gpsimd.collective_compute(
                "AllGather", mybir.AluOpType.bypass, replica_groups=PAIRS,
                ins=[cc_in_kt[kc].opt()], outs=[cc_out_kt[kc].opt()],
            )
            # readback both ranks (own half re-read too: keeps the program
            # rank-symmetric and the key order [rank0|rank1])
            for r in range(2):
                k0 = r * KH + kc * 512
                nc.sync.dma_start(
                    KT[:, :, k0 : k0 + 512],
                    cc_out_kt[kc][r].rearrange("d p x -> p d x"),
                )
        close(wlop_cm)

        # ---- Phase 2: V-own = decT_own^T @ Wkv_hi -> stage -> cc_in -------
        for kt in range(NKTH):
            pss = [psproj.tile([128, 512], F32, tag="psp", name=f"psv{kt}_{i}") for i in range(NDC)]
            kc, ko = (kt * 128) // 512, (kt * 128) % 512
            for di in range(DS):
                for dc in range(NDC):
                    mm = nc.tensor.matmul(
                        pss[dc][:], dec_s[:, kc, di, ko : ko + 128],
                        whi_s[:, di, dc * 512 : (dc + 1) * 512],
                        start=(di == 0), stop=(di == DS - 1),
                    )
                    if ELIDE_LDW and dc > 0:
                        mm.ins.ldweights = False
            for dc in range(NDC):
                st = stgp.tile([128, 512], BF16, tag="stg", name=f"stv{kt}_{dc}")
                nc.vector.tensor_copy(st[:], pss[dc][:])
                nc.sync.dma_start(cc_in_v[kt, dc], st[:])
        nc.gpsimd.collective_compute(
            "AllGather", mybir.AluOpType.bypass, replica_groups=PAIRS,
            ins=[cc_in_v.opt()], outs=[cc_out_v.opt()],
        )
        for r in range(2):
            for dc in range(NDC):
                nc.sync.dma_start(
                    V[:, r * NKTH : (r + 1) * NKTH, dc * 512 : (dc + 1) * 512],
                    cc_out_v[r, :, dc].rearrange("t p x -> p t x"),
                )
        close(stgp_cm)
        close(decp_cm)
        close(whip_cm)

        # ---- Phase 3: QT = Wq^T @ hsT (stays in SBUF) ---------------------
        for do in range(NDO):
            pss = [psproj.tile([128, 512], F32, tag="psp", name=f"psq{do}_{i}") for i in range(NQC)]
            for di in range(DS):
                for qc in range(NQC):
                    mm = nc.tensor.matmul(
                        pss[qc][:], wq_s[:, do, di, :],
                        hs_s[:, di, qc * 512 : (qc + 1) * 512],
                        start=(di == 0), stop=(di == DS - 1),
                    )
                    if ELIDE_LDW and qc > 0:
                        mm.ins.ldweights = False
            for qc in range(NQC):
                nc.vector.tensor_copy(
                    QT[:, do, qc * 512 : (qc + 1) * 512], pss[qc][:]
                )
        close(wqp_cm)
        close(hstp_cm)
        close(psproj_cm)

        # ---- Phase 4: scores^T + exp + row-sum partials -------------------
        ptp = enter(tc.tile_pool(name="ptp", bufs=2, side="right"))
        lap = enter(tc.tile_pool(name="lap", bufs=2, side="right"))
        ps_sc_cm = tc.tile_pool(name="ps_sc", bufs=4, space="PSUM")
        ps_sc = enter(ps_sc_cm)
        PTs = [ptp.tile([128, NKT, 512], BF16, tag="pt", name=f"PT{qc}")
               for qc in range(NQC)]
        Las = [lap.tile([128, 512], F32, tag="la", name=f"La{qc}")
               for qc in range(NQC)]
        for kt in range(NKT):
            pss = [ps_sc.tile([128, 512], F32, tag="ps_sc", name=f"pss{kt}_{i}") for i in range(NQC)]
            for di in range(DS):
                for qc in range(NQC):
                    mm = nc.tensor.matmul(
                        pss[qc][:], KT[:, di, kt * 128 : (kt + 1) * 128],
                        QT[:, di, qc * 512 : (qc + 1) * 512],
                        start=(di == 0), stop=(di == DS - 1),
                    )
                    if ELIDE_LDW and qc > 0:
                        mm.ins.ldweights = False
            for qc in range(NQC):
                nc.scalar.activation(
                    PTs[qc][:, kt, :], pss[qc][:], EXP,
                    bias=0.0, scale=float(scale),
                )
                if kt == 0:
                    nc.vector.tensor_copy(Las[qc][:], PTs[qc][:, kt, :])
                else:
                    nc.vector.tensor_tensor(
                        Las[qc][:], Las[qc][:], PTs[qc][:, kt, :],
                        mybir.AluOpType.add,
                    )

        # ---- Phase 5: AV + normalize per q-tile ---------------------------
        ps_av_cm = tc.tile_pool(name="ps_av", bufs=3, space="PSUM")
        psl_cm = tc.tile_pool(name="psl", bufs=1, space="PSUM")
        ps_av = enter(ps_av_cm)
        psl = enter(psl_cm)
        for qg in range(NQT):
            qc, qt = qg // (NQT // NQC), qg % (NQT // NQC)
            lt = psl.tile([128, 1], F32, tag="psl", name=f"lt{qg}")
            nc.tensor.matmul(
                lt[:], Las[qc][:, qt * 128 : (qt + 1) * 128], ones[:],
                start=True, stop=True,
            )
            rc = rcp.tile([128, 1], F32, tag="rc", name=f"rc{qg}")
            nc.vector.reciprocal(rc[:], lt[:])
            avs = [ps_av.tile([128, 512], F32, tag="ps_av", name=f"av{qg}_{i}") for i in range(NDC)]
            for kt in range(NKT):
                for dc in range(NDC):
                    mm = nc.tensor.matmul(
                        avs[dc][:], PTs[qc][:, kt, qt * 128 : (qt + 1) * 128],
                        V[:, kt, dc * 512 : (dc + 1) * 512],
                        start=(kt == 0), stop=(kt == NKT - 1),
                    )
                    if ELIDE_LDW and dc > 0:
                        mm.ins.ldweights = False
            ot = ostp.tile([128, Dp], F32, tag="ost", name=f"ot{qg}")
            for dc in range(NDC):
                nc.scalar.activation(
                    ot[:, dc * 512 : (dc + 1) * 512], avs[dc][:],
                    ACOPY, bias=0.0, scale=rc[:],
                )
            nc.sync.dma_start(out[qg * 128 : (qg + 1) * 128, :], ot[:])

        for cm in list(reversed(pools)):
            close(cm)

    if ELIDE_LDW:
        elide_redundant_ldweights(nc)
    legalize_waits(nc)
    return nc


def _bf16(x):
    return np.asarray(x, dtype=ml_dtypes.bfloat16)


def prepare_in_maps(hidden_states, decoder_hidden_states, Wq, Wkv):
    hs = np.asarray(hidden_states, dtype=np.float32)
    dec = np.asarray(decoder_hidden_states, dtype=np.float32)
    Wq = np.asarray(Wq, dtype=np.float32)
    Wkv = np.asarray(Wkv, dtype=np.float32)
    QS = QL // 2
    KH = KL // 2
    DS = D // 128
    NDO = D // 128

    # wq/wkvlo: [p, do, di, o] = W[di*128+p, do*128+o]
    def pack_st(W):
        r = W.reshape(DS, 128, NDO, 128).transpose(1, 2, 0, 3)
        return _bf16(np.ascontiguousarray(r))

    wq_p = pack_st(Wq)
    wkvlo_p = pack_st(Wkv[:, :D])
    # wkvhi: [p, di, j] = Wkv[di*128+p, D+j]
    wkvhi_p = _bf16(np.ascontiguousarray(
        Wkv[:, D:].reshape(DS, 128, D).transpose(1, 0, 2)))

    def pack_xT(x):
        # [N, D] -> [p, di, n] = x[n, di*128+p]
        n = x.shape[0]
        r = x.reshape(n, DS, 128).transpose(2, 1, 0)
        return _bf16(np.ascontiguousarray(r))

    in_maps = []
    for c in range(N_CORES):
        b, h = c // 2, c % 2
        dec_own = dec[b, h * KH : (h + 1) * KH]
        in_maps.append({
            "hsT": pack_xT(hs[b, h * QS : (h + 1) * QS]),
            "decT": np.stack([
                pack_xT(dec_own[kc * 512 : (kc + 1) * 512])
                for kc in range(KH // 512)
            ]),
            "wq": wq_p,
            "wkvlo": wkvlo_p,
            "wkvhi": wkvhi_p,
        })
    return in_maps


def kernel(hidden_states, decoder_hidden_states, Wq, Wkv):
    QS = QL // 2
    scale = 1.0 / float(np.sqrt(D))

    nc = bass.Bass()
    build_attention(nc, QS, KL, D, scale)
    in_maps = prepare_in_maps(hidden_states, decoder_hidden_states, Wq, Wkv)

    res = run_bass_kernel_spmd(nc, in_maps, list(range(N_CORES)))

    out = np.empty((B, QL, D), dtype=np.float32)
    for c in range(N_CORES):
        b, h = c // 2, c % 2
        out[b, h * QS : (h + 1) * QS] = res.results[c]["out"]
    return out
